# revision 54
# baseline (speedup 1.0000x reference)
"""Trainium2 Bass kernel for nn_Attention_50500225466997.

Computation (per batch): qkv = BN(conv1x1(x)); 4-head attention over L=1024
(DK=32, DH=64); out = attn + BN(dwconv3x3(v)); y = BN(conv1x1(out)).

Strategy:
  - Data-parallel over batch: 16 batches -> 8 NeuronCores, 2 per core.
  - All BN scales/permutations/SCALE folded into weights on the host.
  - Scores computed TRANSPOSED (S^T[l, m], l on partitions) so softmax
    needs no P-transposes: E = exp(S^T) unnormalized; Z rides FREE in the
    V-matmul by augmenting each head's vT lhsT with a 64-wide ones block
    ([vT_e|1] / [1|vT_o]) -> banks [O_e;Z_e] / [Z_o;O_o].
  - bf16 operands on every matmul (PSUM accumulate stays fp32): enables
    fast weight load (FWL) so per-MM LDWEIGHTS hides under streaming.
  - Scalar engine does ONLY the exp stream (the hard ~73us floor); all
    PSUM drains/bias-adds run on DVE via tensor_scalar with per-partition
    bias APs; softmax 1/Z via reciprocal_approx_fast (~5x cheaper).
  - Depthwise 3x3 via 9 diagonal-matrix matmuls accumulating in PSUM;
    pad images are persistent zeroed tiles, interiors refreshed by DMA.
  - vT transposes merged: one [128,128] PE transpose covers both heads of
    a channel tile (16 instead of 64 transposes per batch).
  - Cross-batch software pipelining: batch b+1's qkv/transposes emitted
    before batch b's depthwise/pointwise tail.
"""

import numpy as np

import concourse.bass as bass
import concourse.mybir as mybir
import concourse.tile as tile
from concourse import bacc
from concourse.bass_utils import run_bass_kernel_spmd

F32 = mybir.dt.float32
F32R = mybir.dt.float32r
BF16 = mybir.dt.bfloat16
AF = mybir.ActivationFunctionType
OP = mybir.AluOpType

B, CH, HH, WW = 16, 256, 32, 32
L = HH * WW                   # 1024
NH, DK, DH = 4, 32, 64
CQKV = CH + DK * NH * 2       # 512
SCALE = DK ** (-0.5)
NCORES = 8
BL = B // NCORES              # batches per core

# precision fallback flags: True -> that stage runs fp32r instead of bf16
CFG = {"qkv32": False, "sc32": False, "v32": False, "pw32": False}


def build_bass():
    nc = bacc.Bacc("TRN2", target_bir_lowering=False, debug=False)
    DT_X = F32R if CFG["qkv32"] else BF16   # x, wqkv
    DT_QK = F32R if CFG["sc32"] else BF16   # Qa/Ka (scores operands)
    DT_V = F32R if CFG["v32"] else BF16     # V0/V1, Vt, Et, diag, pads
    DT_PW = F32R if CFG["pw32"] else BF16   # out2, wpw

    x_d = nc.dram_tensor("x", [BL, CH, L], DT_X, kind="ExternalInput")
    wqkvT_d = nc.dram_tensor("wqkvT", [128, 2, CQKV], DT_X, kind="ExternalInput")
    bqkv_d = nc.dram_tensor("bqkv", [128, 4], F32, kind="ExternalInput")
    wpwT_d = nc.dram_tensor("wpwT", [128, 2, CH], DT_PW, kind="ExternalInput")
    bpw_d = nc.dram_tensor("bpw", [128, 2], F32, kind="ExternalInput")
    diag_d = nc.dram_tensor("diag", [128, 18, 128], DT_V, kind="ExternalInput")
    id128_d = nc.dram_tensor("id128", [128, 128], DT_V, kind="ExternalInput")
    ones_d = nc.dram_tensor("ones512", [128, 512], DT_V, kind="ExternalInput")
    out_d = nc.dram_tensor("out", [BL, CH, L], F32, kind="ExternalOutput")

    with tile.TileContext(nc) as tc, nc.allow_low_precision(reason="bf16"):
        with (
            tc.tile_pool(name="consts", bufs=1) as consts,
            tc.tile_pool(name="xin", bufs=4) as xin,
            tc.tile_pool(name="qkv", bufs=8) as qkvp,
            tc.tile_pool(name="vt", bufs=1) as vtp,
            tc.tile_pool(name="et", bufs=6) as etp,
            tc.tile_pool(name="o2", bufs=4) as o2p,
            tc.tile_pool(name="small", bufs=8) as smallp,
            tc.tile_pool(name="pad", bufs=1) as padp,
            tc.tile_pool(name="outp", bufs=4) as outp,
            tc.tile_pool(name="psc", bufs=2, space="PSUM") as psc,
            tc.tile_pool(name="pO", bufs=2, space="PSUM") as pOp,
            tc.tile_pool(name="pwork", bufs=2, space="PSUM") as pwork,
        ):
            # ------------- constants, spread over idle trigger queues -----
            # sync carries ONLY wqkvT first so the first qkv matmul's weights
            # arrive asap; everything else rides other engines' DMA queues.
            wqkvT = consts.tile([128, 2, CQKV], DT_X)
            nc.sync.dma_start(wqkvT, wqkvT_d.ap())
            bqkv = consts.tile([128, 4], F32)
            nc.scalar.dma_start(bqkv, bqkv_d.ap())
            id128 = consts.tile([128, 128], DT_V)
            nc.scalar.dma_start(id128, id128_d.ap())
            ones512 = consts.tile([128, 8, 64], DT_V)
            nc.scalar.dma_start(ones512, ones_d.ap().rearrange("p (c d) -> p c d", c=8))
            wpwT = consts.tile([128, 2, CH], DT_PW)
            bpw = consts.tile([128, 2], F32)
            diag = consts.tile([128, 18, 128], DT_V)

            # PE warm-up: zero matmuls during the DMA ramp keep the HAM
            # activity monitor at full clock so real matmuls start warm.
            wz = consts.tile([128, 512], DT_X, name="warmzero")
            nc.vector.memzero(wz)
            for wi in range(12):
                pwz = pwork.tile([128, 512], F32, name=f"wz{wi}", tag="w")
                nc.tensor.matmul(pwz, wz[:, 0:128], wz, start=True, stop=True)

            # prefetch all batches' x (xin bufs cover BL*2 tiles)
            Xall = []
            for b in range(BL):
                Xb = []
                for ct in range(2):
                    xt = xin.tile([128, L], DT_X, name=f"x_b{b}c{ct}", tag="x")
                    Xb.append(xt)
                if b == 0:
                    # split batch-0 x across the scalar and gpsimd trigger
                    # queues (both idle at start; sync is busy with wqkvT)
                    for half in range(2):
                        for ct in range(2):
                            hs = slice(512 * half, 512 * half + 512)
                            eng = nc.scalar if ct == 0 else nc.gpsimd
                            eng.dma_start(
                                Xb[ct][:, hs],
                                x_d.ap()[b, 128 * ct : 128 * ct + 128, hs],
                            )
                else:
                    for ct in range(2):
                        nc.sync.dma_start(
                            Xb[ct], x_d.ap()[b, 128 * ct : 128 * ct + 128, :]
                        )
                Xall.append(Xb)

            # persistent vt tiles, one set per in-flight batch; all ones
            # halves written by the idle-at-ramp scalar engine so the DVE
            # queue carries only the exp-gating qkv drains early on.
            Vt = []
            for s in range(2):
                Vs = []
                for h in range(NH):
                    vt_h = vtp.tile([128, 8, 128], DT_V, name=f"vt{h}_{s}")
                    par = h % 2
                    nc.scalar.copy(
                        vt_h[:, :, 64 - 64 * par : 128 - 64 * par], ones512
                    )
                    Vs.append(vt_h)
                Vt.append(Vs)

            # persistent padded-image tiles (34x34), zero-filled once;
            # interiors are DMA-refreshed per batch, edges stay zero.
            # (gpsimd/scalar are idle at start; keep DVE free for drains)
            pads = []
            for s in range(2):
                ps = []
                for ct in range(2):
                    padt = padp.tile([128, 34, 34], DT_V, name=f"pad{s}{ct}")
                    eng = nc.gpsimd if s == 0 else nc.scalar
                    eng.memzero(padt)
                    ps.append(padt)
                pads.append(ps)

            # ---- staged emission with cross-batch software pipelining ----
            st = [{} for _ in range(BL)]

            def emit_head(b, parts=(1, 2)):
                X = Xall[b]
                s = b % 2
                if 1 in parts:
                    st[b]["qkv"] = [
                        qkvp.tile([128, L], DT_QK, name=f"Qa_{b}", tag="Qa"),
                        qkvp.tile([128, L], DT_QK, name=f"Ka_{b}", tag="Ka"),
                        qkvp.tile([128, L], DT_V, name=f"V0_{b}", tag="V0"),
                        qkvp.tile([128, L], DT_V, name=f"V1_{b}", tag="V1"),
                    ]
                dsts = st[b]["qkv"]
                Qa, Ka, V0, V1 = dsts

                def qkv_mm(ot, mt):
                    ms = slice(512 * mt, 512 * mt + 512)
                    pq = pwork.tile([128, 512], F32, name=f"pq{b}{mt}{ot}", tag="w")
                    for kt in range(2):
                        nc.tensor.matmul(
                            pq,
                            wqkvT[:, kt, 128 * ot : 128 * ot + 128],
                            X[kt][:, ms],
                            start=(kt == 0),
                            stop=(kt == 1),
                        )
                    nc.vector.tensor_scalar_add(
                        dsts[ot][:, ms], pq, bqkv[:, ot : ot + 1]
                    )

                def emit_vt(ct):
                    # vT transposes: one [128,128] block covers both heads
                    # of a channel tile; psum halves go to the augmented Vt
                    vsrc = [V0, V1][ct]
                    he, ho = 2 * ct, 2 * ct + 1
                    for half in range(2):
                        pv = pwork.tile(
                            [128, 4, 128], DT_V, name=f"pv{b}{ct}{half}", tag="w"
                        )
                        for c4 in range(4):
                            c8 = 4 * half + c4
                            nc.tensor.transpose(
                                pv[:, c4, :],
                                vsrc[:, 128 * c8 : 128 * c8 + 128],
                                id128,
                            )
                        lts = slice(4 * half, 4 * half + 4)
                        # head-even v-dims sit in cols 0:64 (aug cols 0:64),
                        # head-odd in cols 64:128 (aug cols 64:128).
                        # Only the earliest-needed block (batch 0, ct0,
                        # lt 0-3) rides the idle scalar queue; the rest stay
                        # on DVE so neither FIFO gates the exp stream.
                        if b == 0 and ct == 0 and half == 0:
                            nc.scalar.copy(
                                Vt[s][he][:, lts, 0:64], pv[:, :, 0:64]
                            )
                            nc.scalar.copy(
                                Vt[s][ho][:, lts, 64:128], pv[:, :, 64:128]
                            )
                        else:
                            nc.vector.tensor_copy(
                                Vt[s][he][:, lts, 0:64], pv[:, :, 0:64]
                            )
                            nc.vector.tensor_copy(
                                Vt[s][ho][:, lts, 64:128], pv[:, :, 64:128]
                            )
                    # pad interior refresh (edges stay zero) off gpsimd queue
                    nc.gpsimd.dma_start(
                        pads[s][ct][:, 1:33, 1:33],
                        vsrc.rearrange("p (a c) -> p a c", a=32),
                    )

                # Ka fully + Qa-mt0 first (attention mt0 starts then); V
                # drains + transposes next (bank matmuls need Vt early);
                # Qa-mt1 last (only needed ~10us later by the mt1 pass).
                if 1 in parts:
                    qkv_mm(1, 0)
                    qkv_mm(1, 1)
                    qkv_mm(0, 0)
                    qkv_mm(2, 0)
                    qkv_mm(2, 1)
                    if b == 0:
                        # late consts: needed only by dw/pw phases
                        nc.sync.dma_start(wpwT, wpwT_d.ap())
                        nc.sync.dma_start(bpw, bpw_d.ap())
                        nc.sync.dma_start(diag, diag_d.ap())
                    emit_vt(0)
                if 2 in parts:
                    qkv_mm(3, 0)
                    qkv_mm(3, 1)
                    emit_vt(1)
                    qkv_mm(0, 1)
                    out2 = [
                        o2p.tile([128, L], DT_PW, name=f"o2_{b}{ct}", tag="o2")
                        for ct in range(2)
                    ]
                    st[b].update(Qa=Qa, Ka=Ka, out2=out2)

            def emit_attn(b, hp, mts=(0, 1)):
                # per (hp, mt): bank A = [vT_e|1].T @ E_e = [O_e; Z_e],
                # bank B = [1|vT_o].T @ E_o = [Z_o; O_o]; 1/Z via two
                # fast-approx reciprocals; halves swapped with SBUF DMA.
                s = b % 2
                Qa, Ka, out2 = st[b]["Qa"], st[b]["Ka"], st[b]["out2"]
                for mt in mts:
                    ms = slice(512 * mt, 512 * mt + 512)
                    pA = pOp.tile([128, 512], F32, name=f"pa{b}{hp}{mt}", tag="o")
                    pB = pOp.tile([128, 512], F32, name=f"pb{b}{hp}{mt}", tag="o")
                    banks = [pA, pB]
                    # software-pipelined: scores run one lt AHEAD of the
                    # bank matmuls, so the FIFO PE queue never head-of-line
                    # blocks on exp(lt) while scores(lt+1) could run.
                    Ets = [None] * 8
                    for lt in range(9):
                        if lt < 8:
                            ls = slice(128 * lt, 128 * lt + 128)
                            sc = psc.tile(
                                [128, 1024], F32, name=f"sc{b}{hp}{mt}{lt}", tag="sc"
                            )
                            for j in range(2):
                                h = 2 * hp + j
                                nc.tensor.matmul(
                                    sc[:, 512 * j : 512 * j + 512],
                                    Ka[32 * h : 32 * h + 32, ls],
                                    Qa[32 * h : 32 * h + 32, ms],
                                    start=True,
                                    stop=True,
                                    tile_position=(32 * h, 0),
                                )
                            Et = etp.tile(
                                [128, 1024], DT_V, name=f"e{b}{hp}{mt}{lt}", tag="e"
                            )
                            nc.scalar.activation(Et, sc, AF.Exp)
                            Ets[lt] = Et
                        if lt >= 1:
                            k = lt - 1
                            for j in range(2):
                                h = 2 * hp + j
                                nc.tensor.matmul(
                                    banks[j],
                                    Vt[s][h][:, k, :],
                                    Ets[k][:, 512 * j : 512 * j + 512],
                                    start=(k == 0),
                                    stop=(k == 7),
                                    skip_group_check=True,
                                )
                    # stage Z AND O halves to SBUF with 4 in-lane copies so
                    # the pO banks release fast; pA's two copies come FIRST
                    # so its bank frees for the next iteration's first bank
                    # matmul (pO pool cycles pA->next pB).
                    ZA = smallp.tile([128, 512], F32, name=f"za{b}{hp}{mt}", tag="za")
                    Ou = smallp.tile([128, 512], F32, name=f"ou{b}{hp}{mt}", tag="ou")
                    nc.vector.tensor_copy(ZA[64:128, :], pA[64:128, :])
                    nc.vector.tensor_copy(Ou[0:64, :], pA[0:64, :])
                    nc.vector.tensor_copy(ZA[0:64, :], pB[0:64, :])
                    nc.vector.tensor_copy(Ou[64:128, :], pB[64:128, :])
                    rf = smallp.tile([128, 512], F32, name=f"rf{b}{hp}{mt}", tag="rf")
                    nc.vector.reciprocal_approx_fast(out=rf, in_=ZA)
                    rz = smallp.tile([128, 512], F32, name=f"rz{b}{hp}{mt}", tag="rz")
                    nc.sync.dma_start(rz[0:64, :], rf[64:128, :])
                    nc.sync.dma_start(rz[64:128, :], rf[0:64, :])
                    nc.vector.scalar_tensor_tensor(
                        out=out2[hp][:, ms],
                        in0=Ou,
                        scalar=1.0,
                        in1=rz,
                        op0=OP.mult,
                        op1=OP.mult,
                    )

            def emit_dw(b, ct, mts=(0, 1)):
                s = b % 2
                padt, out2 = pads[s][ct], st[b]["out2"]
                for mt in mts:
                    ms = slice(512 * mt, 512 * mt + 512)
                    dwp = pwork.tile([128, 512], F32, name=f"dw{b}{ct}{mt}", tag="w")
                    for tap in range(9):
                        dy, dx = tap // 3, tap % 3
                        r0 = 16 * mt + dy
                        nc.tensor.matmul(
                            dwp,
                            diag[:, 9 * ct + tap, :],
                            padt[:, r0 : r0 + 16, dx : dx + 32],
                            start=(tap == 0),
                            stop=(tap == 8),
                        )
                    nc.vector.scalar_tensor_tensor(
                        out=out2[ct][:, ms],
                        in0=dwp,
                        scalar=1.0,
                        in1=out2[ct][:, ms],
                        op0=OP.mult,
                        op1=OP.add,
                    )

            def emit_pw(b, mt):
                out2 = st[b]["out2"]
                ms = slice(512 * mt, 512 * mt + 512)
                for ot in range(2):
                    pp = pwork.tile([128, 512], F32, name=f"pp{b}{mt}{ot}", tag="w")
                    for kt in range(2):
                        nc.tensor.matmul(
                            pp,
                            wpwT[:, kt, 128 * ot : 128 * ot + 128],
                            out2[kt][:, ms],
                            start=(kt == 0),
                            stop=(kt == 1),
                        )
                    osb = outp.tile([128, 512], F32, name=f"os{b}{mt}{ot}", tag="os")
                    nc.vector.tensor_scalar_add(osb, pp, bpw[:, ot : ot + 1])
                    nc.gpsimd.dma_start(
                        out_d.ap()[b, 128 * ot : 128 * ot + 128, ms], osb
                    )

            # pw(b, mt) waits on the full attention normalize chain of its
            # batch; since each engine queue is FIFO, pw must sit BEHIND
            # independent attention matmuls or it head-of-line-blocks the PE.
            # pw(b, mt) waits on the full attention normalize chain of its
            # batch; since each engine queue is FIFO, pw must sit BEHIND
            # independent attention matmuls or it head-of-line-blocks the PE.
            assert BL == 2
            emit_head(0)
            emit_attn(0, 0)
            emit_dw(0, 0)
            emit_attn(0, 1)
            emit_head(1)          # batch-1 qkv before batch-0 tail work
            emit_dw(0, 1)
            emit_attn(1, 0, mts=(0,))
            emit_pw(0, 0)
            emit_attn(1, 0, mts=(1,))
            emit_pw(0, 1)
            emit_dw(1, 0)
            emit_attn(1, 1, mts=(0,))
            emit_dw(1, 1, mts=(0,))
            emit_pw(1, 0)
            emit_attn(1, 1, mts=(1,))
            # keep the PE warm+busy through the final normalize chain so the
            # last depthwise/pointwise matmuls don't run at throttled clock
            for wi in range(10):
                pwz = pwork.tile([128, 512], F32, name=f"tz{wi}", tag="w")
                nc.tensor.matmul(pwz, wz[:, 0:128], wz, start=True, stop=True)
            emit_dw(1, 1, mts=(1,))
            emit_pw(1, 1)

    nc.compile()
    return nc


def pack_inputs(w_qkv, s_qkv, b_qkv, w_dw, s_dw, b_dw, w_pw, s_pw, b_pw):
    """Host-side weight packing. Returns dict of constant arrays (shared by
    all cores)."""
    import ml_dtypes

    f32 = np.float32
    bf = np.float32 if CFG["qkv32"] else ml_dtypes.bfloat16
    bfv = np.float32 if CFG["v32"] else ml_dtypes.bfloat16
    bfp = np.float32 if CFG["pw32"] else ml_dtypes.bfloat16

    Wq = (w_qkv[:, :, 0, 0] * s_qkv[:, None]).astype(np.float64)  # [512, 256]
    bq = b_qkv.astype(np.float64).copy()

    # permute output channels to [Q_all, K_all, V0, V1]
    perm = []
    for h in range(NH):
        perm += [h * 128 + d for d in range(32)]           # q
    for h in range(NH):
        perm += [h * 128 + 32 + d for d in range(32)]      # k
    for h in range(NH):
        perm += [h * 128 + 64 + d for d in range(64)]      # v
    perm = np.array(perm)
    Wq = Wq[perm]
    bq = bq[perm]
    # fold attention scale into q
    Wq[0:128] *= SCALE
    bq[0:128] *= SCALE

    wqkvT = np.ascontiguousarray(
        Wq.T.reshape(2, 128, CQKV).transpose(1, 0, 2)
    ).astype(bf)  # [128, 2, 512]
    bqkv = np.ascontiguousarray(bq.reshape(4, 128).T).astype(f32)  # [128, 4]

    Wp = (w_pw[:, :, 0, 0] * s_pw[:, None]).astype(np.float64)     # [256, 256]
    bp = b_pw.astype(np.float64) + Wp @ b_dw.astype(np.float64)
    wpwT = np.ascontiguousarray(
        Wp.T.reshape(2, 128, CH).transpose(1, 0, 2)
    ).astype(bfp)  # [128, 2, 256]
    bpw = np.ascontiguousarray(bp.reshape(2, 128).T).astype(f32)   # [128, 2]

    wd = (w_dw[:, 0] * s_dw[:, None, None]).astype(f32)            # [256, 3, 3]
    diag = np.zeros((128, 18, 128), f32)
    for ct in range(2):
        for tap in range(9):
            dy, dx = tap // 3, tap % 3
            idx = np.arange(128)
            diag[idx, 9 * ct + tap, idx] = wd[128 * ct + idx, dy, dx]

    return {
        "wqkvT": wqkvT,
        "bqkv": bqkv,
        "wpwT": wpwT,
        "bpw": bpw,
        "diag": diag.astype(bfv),
        "id128": np.eye(128, dtype=f32).astype(bfv),
        "ones512": np.ones((128, 512), bfv),
    }


_NC_CACHE = None


def _get_nc():
    global _NC_CACHE
    if _NC_CACHE is None:
        _NC_CACHE = build_bass()
    return _NC_CACHE


def run(inputs, trace=False):
    """Run the bass kernel on 8 cores. inputs = the reference input dict.
    Returns (full_output [16,256,32,32], BassKernelResults)."""
    import ml_dtypes

    xdt = np.float32 if CFG["qkv32"] else ml_dtypes.bfloat16
    x = np.ascontiguousarray(
        np.asarray(inputs["x"], dtype=np.float32).reshape(B, CH, L)
    ).astype(xdt)
    consts = pack_inputs(
        np.asarray(inputs["w_qkv"], np.float32),
        np.asarray(inputs["s_qkv"], np.float32),
        np.asarray(inputs["b_qkv"], np.float32),
        np.asarray(inputs["w_dw"], np.float32),
        np.asarray(inputs["s_dw"], np.float32),
        np.asarray(inputs["b_dw"], np.float32),
        np.asarray(inputs["w_pw"], np.float32),
        np.asarray(inputs["s_pw"], np.float32),
        np.asarray(inputs["b_pw"], np.float32),
    )
    in_maps = []
    for c in range(NCORES):
        m = dict(consts)
        m["x"] = np.ascontiguousarray(x[c * BL : (c + 1) * BL])
        in_maps.append(m)

    nc = _get_nc()
    res = run_bass_kernel_spmd(
        nc, in_maps, core_ids=list(range(NCORES)), trace=trace
    )
    out = np.concatenate([r["out"] for r in res.results], axis=0)
    return out.reshape(B, CH, HH, WW), res


def kernel(**inputs) -> np.ndarray:
    out, _ = run(inputs, trace=False)
    return out
